# revision 1
# baseline (speedup 1.0000x reference)
"""DualAttention Trainium2 kernel (8 NeuronCores, data-parallel over batch).

Math (per batch b, head h, dk=64, S=1024):
  s   = (q @ k^T) / 8                      [S, S]
  E   = exp(s) with strict-causal mask (j < i) applied as -1e30 pre-exp
  Z1  = rowsum(E)                          (row 0: Z1=0 -> handled specially)
  p1  = (E / Z1) * notcm                   (notcm zeroes counter-masked key cols)
  E2  = exp(p1)  -- dense: exp(0)=1 for all masked/future cols
  Z2  = sum_j E2[j] = rowsum_window(E2) + (S - W)
  out = (E2 @ v)/Z2 = (E2|window @ v|window + colsum_{j>=W} v)/Z2
  row 0 of out is forced to 0 (reference zeroes p row 0 post-softmax).

Kernel strategy per core (1 batch item): loop 8 head-pairs; per head:
scores via PE (bf16), causal -1e30 via a const matmul addend, exp1 on ACT
with fused accum -> Z1, counter-mask+1/Z1 fused in one DVE
scalar_tensor_tensor, one batched exp2 on ACT, DMA-xbar transposes of E2
(bf16) for the P@V matmuls, rank-8 suffix-colsum correction as a K=8
matmul, Z2 via a ones-column matmul, final 1/Z2 on DVE.
"""

import numpy as np

import concourse.bass as bass
import concourse.mybir as mybir
from concourse.tile import TileContext
from concourse.alu_op_type import AluOpType

F32 = mybir.dt.float32
BF16 = mybir.dt.bfloat16

B, S, D = 8, 1024, 1024
H, DK = 16, 64
NCORES = 8
P = 128          # partition block
NQB = S // P     # 8 query blocks
MASKADD = -1e30
# packed offsets for the causal windows W=(qb+1)*128
OFF = [0]
for _qb in range(NQB):
    OFF.append(OFF[-1] + (_qb + 1) * P)
TOTW = OFF[-1]   # 4608


def build_nc():
    from concourse.bacc import Bacc

    nc = Bacc()
    # host passes q/k pre-transposed [D, S] and everything pre-cast to bf16
    qt_d = nc.declare_dram_parameter("qT", [D, S], BF16, isOutput=False)
    kt_d = nc.declare_dram_parameter("kT", [D, S], BF16, isOutput=False)
    v1_d = nc.declare_dram_parameter("v1", [S, D], BF16, isOutput=False)
    v2_d = nc.declare_dram_parameter("v2", [S, D], BF16, isOutput=False)
    cm_d = nc.declare_dram_parameter("cm", [1, S], F32, isOutput=False)
    o1_d = nc.declare_dram_parameter("out1", [S, D], F32, isOutput=True)
    o2_d = nc.declare_dram_parameter("out2", [S, D], F32, isOutput=True)

    from contextlib import ExitStack

    with TileContext(nc) as tc, ExitStack() as ctx:
        const = ctx.enter_context(tc.tile_pool(name="const", bufs=1))
        qkpool = ctx.enter_context(tc.tile_pool(name="qk", bufs=2))
        hpool = ctx.enter_context(tc.tile_pool(name="hp", bufs=3))
        epool = ctx.enter_context(tc.tile_pool(name="ep", bufs=16))
        packp = ctx.enter_context(tc.tile_pool(name="pk", bufs=2))
        etp = ctx.enter_context(tc.tile_pool(name="et", bufs=2))
        smol = ctx.enter_context(tc.tile_pool(name="sm", bufs=6))
        outp = ctx.enter_context(tc.tile_pool(name="op", bufs=2))
        bigp = ctx.enter_context(tc.tile_pool(name="big", bufs=1))
        # PSUM budget (8 banks): ps 2x2 + po 2 + small 2x1
        ps_pool = ctx.enter_context(tc.tile_pool(name="ps", bufs=2, space="PSUM"))
        po_pool = ctx.enter_context(tc.tile_pool(name="po", bufs=1, space="PSUM"))
        pc_pool = ctx.enter_context(tc.tile_pool(name="pc", bufs=2, space="PSUM"))

        # ---------------- constants ----------------
        # touch Exp immediately so the ~2.7us ACT table load overlaps the
        # first input DMAs instead of stalling the first exp1
        warm = const.tile([1, 1], F32, tag="warm")
        nc.gpsimd.memset(warm[:], 0.0)
        nc.scalar.activation(out=warm[:], in_=warm[:],
                             func=mybir.ActivationFunctionType.Exp)

        ident = const.tile([P, P], BF16, tag="ident")
        nc.gpsimd.memset(ident[:], 0.0)
        nc.gpsimd.affine_select(
            out=ident[:], in_=ident[:], compare_op=AluOpType.not_equal,
            fill=1.0, base=0, pattern=[[-1, P]], channel_multiplier=1)

        # tric[r, c] = -1e30 where c >= r (strict causal: only j < i survives)
        # keep 0 where r - c - 1 >= 0 (c < r), else fill -1e30 (c >= r)
        tric = const.tile([P, P], BF16, tag="tric")
        nc.gpsimd.memset(tric[:], 0.0)
        nc.gpsimd.affine_select(
            out=tric[:], in_=tric[:], compare_op=AluOpType.is_ge,
            fill=MASKADD, base=-1, pattern=[[-1, P]], channel_multiplier=1)

        # onehot: 8 blocks [128, 8]; block c has column c all-ones
        onehot = const.tile([P, 64], BF16, tag="onehot")
        nc.gpsimd.memset(onehot[:], 0.0)
        for c in range(NQB):
            nc.gpsimd.memset(onehot[:, c * 8 + c : c * 8 + c + 1], 1.0)

        # stairs[c, qb*128 + j] = 1 where c > qb  (suffix-sum selector)
        # condition c > floor(x/128)  <=>  128*c - x - 1 >= 0
        stairs = const.tile([NQB, S], BF16, tag="stairs")
        nc.gpsimd.memset(stairs[:], 1.0)
        nc.gpsimd.affine_select(
            out=stairs[:], in_=stairs[:], compare_op=AluOpType.is_ge,
            fill=0.0, base=-1, pattern=[[-1, S]], channel_multiplier=P)

        ones_col = const.tile([P, 1], BF16, tag="onescol")
        nc.gpsimd.memset(ones_col[:], 1.0)

        # wconst[:, qb] = S - (qb+1)*128  (the "+(S-W)" part of Z2)
        wconst = const.tile([P, NQB], F32, tag="wconst")
        for qb in range(NQB):
            nc.gpsimd.memset(wconst[:, qb : qb + 1], float(S - (qb + 1) * P))

        # ------------- counter-mask broadcast [128, S] (bf16) -------------
        cmrow = const.tile([1, S], F32, tag="cmrow")
        nc.sync.dma_start(out=cmrow[:], in_=cm_d[:])
        cmrow16 = const.tile([1, S], BF16, tag="cmrow16")
        nc.gpsimd.tensor_copy(cmrow16[:], cmrow[:])
        ones_row16 = const.tile([1, P], BF16, tag="onesrow16")
        nc.gpsimd.memset(ones_row16[:], 1.0)
        cmb = const.tile([P, S], BF16, tag="cmb")
        ps_cm = ps_pool.tile([P, S], F32, tag="ps")
        for half in range(2):
            sl = slice(half * 512, (half + 1) * 512)
            nc.tensor.matmul(ps_cm[:, sl], ones_row16[:], cmrow16[:, sl],
                             start=True, stop=True)
        nc.vector.tensor_copy(cmb[:], ps_cm[:])

        # ------------- main loop: 16 heads, 3-stage software pipeline ------
        # A(h): scores + causal + exp1 (+loads, colsums). B(h): 1/Z1, cmmul,
        # exp2, transpose, P@V. C(h): 1/Z2, scale, store. Emitting
        # A(h), C(h-2), B(h-1) keeps each engine's FIFO free of stalls.
        state = {}
        # full outputs accumulate in SBUF; flushed in 1KB-run DMAs per
        # 4-head group (strided 256B-row writes are ~4x slower)
        big1 = bigp.tile([P, NQB * S], F32, tag="big1")
        big2 = bigp.tile([P, NQB * S], F32, tag="big2")

        def stage_load(hp):
            if hp >= NQB or ("pair", hp) in state:
                return
            dsl = slice(hp * P, (hp + 1) * P)
            qT2 = qkpool.tile([P, S], BF16, tag="qT2")
            kT2 = qkpool.tile([P, S], BF16, tag="kT2")
            nc.sync.dma_start(out=qT2[:], in_=qt_d[dsl, :])
            nc.sync.dma_start(out=kT2[:], in_=kt_d[dsl, :])
            # v tiles: SBUF[p, (c,d)] = DRAM[c*128+p, d], one DMA each
            v1b = hpool.tile([P, S], BF16, tag="v1b")
            v2b = hpool.tile([P, S], BF16, tag="v2b")
            for t_sb, t_dr in ((v1b, v1_d), (v2b, v2_d)):
                nc.sync.dma_start(
                    out=t_sb.rearrange("p (c d) -> p c d", c=NQB),
                    in_=t_dr[:, dsl].rearrange("(c s) d -> s c d", c=NQB))
            state[("pair", hp)] = (qT2, kT2, v1b, v2b)

        def stage_a(h):
            hp, hl = divmod(h, 2)
            stage_load(hp)
            qT2, kT2, v1b, v2b = state[("pair", hp)]
            pb = hl * DK  # partition base of this head inside the pair

            # chunk colsums of [v1|v2] -> cs_sb [8, 128] (bf16). One start
            # marks the whole psum bank pending-zero: first touch of each
            # byte overwrites, later touches accumulate.
            pcs = pc_pool.tile([P, P], F32, tag="small")
            for c in range(NQB):
                lhs = onehot[:, c * 8 : (c + 1) * 8]
                va16 = v1b[:, c * P + pb : c * P + pb + DK]
                vb16 = v2b[:, c * P + pb : c * P + pb + DK]
                nc.tensor.matmul(pcs[0:NQB, 0:DK], lhs, va16,
                                 start=(c == 0), stop=False)
                nc.tensor.matmul(pcs[0:NQB, DK:P], lhs, vb16,
                                 start=False, stop=(c == NQB - 1))
            cs_sb = smol.tile([NQB, P], BF16, tag="cs")
            nc.vector.tensor_copy(cs_sb[:], pcs[0:NQB, :])

            z1 = smol.tile([P, NQB], F32, tag="z1")
            etiles = []
            state[h] = dict(pb=pb, v1b=v1b, v2b=v2b, cs_sb=cs_sb,
                            z1=z1, etiles=etiles)
            _score_exp1(h, range(0, NQB // 2))

        def _score_exp1(h, qbs):
            st = state[h]
            hp, hl = divmod(h, 2)
            qT2, kT2, _, _ = state[("pair", hp)]
            pb, z1, etiles = st["pb"], st["z1"], st["etiles"]
            for qb in qbs:
                W = (qb + 1) * P
                ps = ps_pool.tile([P, S], F32, tag="ps")
                # split at the 512-col psum bank boundary
                for lo in range(0, W, 512):
                    hi = min(lo + 512, W)
                    nc.tensor.matmul(
                        ps[:, lo:hi],
                        qT2[pb : pb + DK, qb * P : (qb + 1) * P],
                        kT2[pb : pb + DK, lo:hi],
                        start=True, stop=(hi < W))
                # add -1e30 to j >= i inside the diagonal block
                nc.tensor.matmul(
                    ps[:, W - P : W], ident[:], tric[:],
                    start=False, stop=True)
                e_t = epool.tile([P, S], BF16, tag="E")
                nc.scalar.activation(
                    out=e_t[:, 0:W], in_=ps[:, 0:W],
                    func=mybir.ActivationFunctionType.Exp,
                    scale=0.125, accum_out=z1[:, qb : qb + 1])
                etiles.append(e_t)

        def stage_a2(h):
            _score_exp1(h, range(NQB // 2, NQB))

        def stage_b1(h):
            st = state[h]
            r1 = smol.tile([P, NQB], F32, tag="r1")
            nc.vector.reciprocal(r1[:], st["z1"][:])
            if True:  # query row 0 has Z1=0; force scale 0 (out row zeroed)
                nc.gpsimd.memset(r1[0:1, 0:1], 0.0)

            # fused (E * 1/Z1) * notcm -> packed pp; exp2 runs in place
            pp = packp.tile([P, TOTW], BF16, tag="pp")
            for qb in range(NQB):
                W = (qb + 1) * P
                nc.vector.scalar_tensor_tensor(
                    out=pp[:, OFF[qb] : OFF[qb] + W],
                    in0=st["etiles"][qb][:, 0:W],
                    scalar=r1[:, qb : qb + 1],
                    in1=cmb[:, 0:W],
                    op0=AluOpType.mult, op1=AluOpType.mult)
            st["pp"] = pp

        def stage_b2(h):
            st = state[h]
            pb, v1b, v2b, pp = st["pb"], st["v1b"], st["v2b"], st["pp"]

            # exp2 in place, split so early query blocks unblock transposes
            nc.scalar.activation(out=pp[:, 0 : OFF[4]], in_=pp[:, 0 : OFF[4]],
                                 func=mybir.ActivationFunctionType.Exp)
            nc.scalar.activation(out=pp[:, OFF[4] :], in_=pp[:, OFF[4] :],
                                 func=mybir.ActivationFunctionType.Exp)

            # all 36 (qb, kc) chunks transposed in two blocked DMAs
            e2t = etp.tile([P, TOTW], BF16, tag="e2t")
            NB4 = OFF[4] // P  # 10 chunks in qb 0..3
            nc.sync.dma_start(
                out=e2t[:, 0 : OFF[4]].rearrange("p (n s) -> p n s", n=NB4),
                in_=pp[:, 0 : OFF[4]].rearrange("p (n s) -> p n s", n=NB4),
                transpose=True)
            nc.sync.dma_start(
                out=e2t[:, OFF[4] :].rearrange("p (n s) -> p n s",
                                               n=TOTW // P - NB4),
                in_=pp[:, OFF[4] :].rearrange("p (n s) -> p n s",
                                              n=TOTW // P - NB4),
                transpose=True)

            # P@[v1|v2] + Z2 ones-column + suffix correction
            po = po_pool.tile([P, S], F32, tag="po")
            pz = pc_pool.tile([P, P], F32, tag="small")
            for qb in range(NQB):
                for kc in range(qb + 1):
                    n = OFF[qb] // P + kc
                    lhs = e2t[:, n * P : (n + 1) * P]
                    va = v1b[:, kc * P + pb : kc * P + pb + DK]
                    vb = v2b[:, kc * P + pb : kc * P + pb + DK]
                    first_bank = kc == 0 and qb % 4 == 0
                    last_bank = qb == NQB - 1 and kc == qb
                    nc.tensor.matmul(po[:, qb * P : qb * P + DK], lhs, va,
                                     start=first_bank, stop=False)
                    nc.tensor.matmul(po[:, qb * P + DK : (qb + 1) * P],
                                     lhs, vb, start=False, stop=last_bank)
                    nc.tensor.matmul(pz[:, qb : qb + 1], lhs, ones_col[:],
                                     start=(qb == 0 and kc == 0),
                                     stop=(qb == NQB - 1 and kc == qb))
                if qb < NQB - 1:
                    # += sum_{keys >= W} v  (rank-8 via stairs selector)
                    nc.tensor.matmul(
                        po[:, qb * P : (qb + 1) * P],
                        stairs[:, qb * P : (qb + 1) * P], st["cs_sb"][:],
                        start=False, stop=(qb == 3))
            st.update(po=po, pz=pz)

        def stage_c(h):
            st = state.pop(h)
            po, pz = st["po"], st["pz"]
            z2 = smol.tile([P, NQB], F32, tag="z2")
            r2 = smol.tile([P, NQB], F32, tag="r2")
            nc.vector.tensor_tensor(
                out=z2[:], in0=pz[0:P, 0:NQB], in1=wconst[:],
                op=AluOpType.add)
            nc.vector.reciprocal(r2[:], z2[:])

            obuf = outp.tile([P, S], F32, tag="osb")
            for qb in range(NQB):
                nc.vector.tensor_scalar_mul(
                    obuf[:, qb * P : (qb + 1) * P],
                    po[:, qb * P : (qb + 1) * P],
                    r2[:, qb : qb + 1])
            # spread into the big output accumulators (gpsimd is idle)
            ob3 = obuf.rearrange("p (c x) -> p c x", c=NQB)
            b13 = big1.rearrange("p (c d) -> p c d", c=NQB)
            b23 = big2.rearrange("p (c d) -> p c d", c=NQB)
            hc = slice(h * DK, (h + 1) * DK)
            nc.gpsimd.tensor_copy(b13[:, :, hc], ob3[:, :, 0:DK])
            nc.gpsimd.tensor_copy(b23[:, :, hc], ob3[:, :, DK:P])
            nc.gpsimd.memset(big1[0:1, h * DK : (h + 1) * DK], 0.0)
            nc.gpsimd.memset(big2[0:1, h * DK : (h + 1) * DK], 0.0)
            if h % 2 == 1:
                g = slice((h - 1) * DK, (h + 1) * DK)
                nc.sync.dma_start(
                    out=o1_d[:, g].rearrange("(c s) d -> s c d", c=NQB),
                    in_=b13[:, :, g])
                nc.sync.dma_start(
                    out=o2_d[:, g].rearrange("(c s) d -> s c d", c=NQB),
                    in_=b23[:, :, g])

        for it in range(H + 2):
            if it < H:
                stage_a(it)
                if it % 2 == 0:
                    stage_load(it // 2 + 1)  # prefetch next pair's inputs
                stage_a2(it)
            if it >= 2:
                stage_c(it - 2)
            if 1 <= it <= H:
                stage_b1(it - 1)
                stage_b2(it - 1)
    nc.compile()
    return nc


_NC_CACHE = None


def _get_nc():
    global _NC_CACHE
    if _NC_CACHE is None:
        _NC_CACHE = build_nc()
    return _NC_CACHE


def prep_inputs(q, k, v1, v2, counter_attention_mask):
    """Host-side shard prep: transpose q/k per batch, cast all to bf16."""
    import ml_dtypes

    bf = ml_dtypes.bfloat16
    q = np.asarray(q, dtype=np.float32)
    k = np.asarray(k, dtype=np.float32)
    v1 = np.asarray(v1, dtype=np.float32).astype(bf)
    v2 = np.asarray(v2, dtype=np.float32).astype(bf)
    cm = np.asarray(counter_attention_mask)
    notcm = (cm == 0).astype(np.float32)  # [B, S]
    return [
        {"qT": np.ascontiguousarray(q[b].astype(bf).T),
         "kT": np.ascontiguousarray(k[b].astype(bf).T),
         "v1": v1[b], "v2": v2[b],
         "cm": notcm[b : b + 1, :]}
        for b in range(NCORES)
    ]


def kernel(q, k, v1, v2, counter_attention_mask):
    from concourse.bass_utils import run_bass_kernel_spmd

    in_maps = prep_inputs(q, k, v1, v2, counter_attention_mask)
    nc = _get_nc()
    res = run_bass_kernel_spmd(nc, in_maps, list(range(NCORES))).results
    out1 = np.stack([res[b]["out1"] for b in range(NCORES)])
    out2 = np.stack([res[b]["out2"] for b in range(NCORES)])
    return out1, out2



# revision 4
# speedup vs baseline: 1.3498x; 1.3498x over previous
"""DualAttention Trainium2 kernel v2 (8 NeuronCores, data-parallel over batch).

Math per batch/head (dk=64, S=1024), with per-chunk key permutation putting
counter-mask-unmasked keys first (host-side):
  E   = exp(q@kp^T/8) windowed (strict causal via per-batch permuted tric)
  Z1  = rowsum(E) over the full window (ACT accum)
  p1  = (E/Z1)*w on the FP-wide packed prefix of each 128-chunk only
        (FP chosen at runtime >= max unmasked/chunk; w zeroes masked/pad
        slots, pad cols hold -100 so exp2 -> 0)
  E2  = exp(p1) on packed groups (padded to 128-aligned per-qb groups)
  out = (E2 @ [vp1|vp2|1] + ones x addrow_qb) / Z2
        addrow_qb = [colsum(v) - colsum(vp[:L]), S - L]  (host-precomputed;
        folds the E2=1 background for masked/future keys AND Z2's constant)
  Z2  = aug column of the same PSUM accumulation. Row 0 forced to 0.

vs v1: ~-30% exp2/mask/transpose/PV work, ~60 fewer small matmuls per head
(pz/stairs/colsum machinery folded into the augmented matmul + host rows),
bf16 outputs (half the output DMA traffic), DVE queue no longer blocked by
the 1/Z2 reciprocals (B1 emitted before C).
"""

import numpy as np

import concourse.bass as bass
import concourse.mybir as mybir
from concourse.tile import TileContext
from concourse.alu_op_type import AluOpType

F32 = mybir.dt.float32
BF16 = mybir.dt.bfloat16

B, S, D = 8, 1024, 1024
H, DK = 16, 64
NCORES = 8
P = 128          # partition block
NQB = S // P     # 8 query blocks
AUGW = 130       # per-qb PV output block: v1(64) | v2(64) | z2(1) | pad(1)
MASKADD = -1e30
PADVAL = -100.0  # exp2(pad) == 0
POW = 1536       # po: 3 qb-blocks of AUGW per 512-f32 PSUM bank


class Layout:
    """Packed-prefix layout for a given prefix width FP (multiple of 16)."""

    def __init__(self, fp):
        self.FP = fp
        self.NPK = NQB * fp
        self.LREAL = [fp * (qb + 1) for qb in range(NQB)]
        self.LPAD = [((l + P - 1) // P) * P for l in self.LREAL]
        self.OFFP = [0]
        for l in self.LPAD:
            self.OFFP.append(self.OFFP[-1] + l)
        self.TOTP = self.OFFP[-1]
        # exp2/transpose split point: group boundary near the middle,
        # 128-chunk aligned
        mid = min(range(1, NQB), key=lambda i: abs(self.OFFP[i] - self.TOTP // 2))
        self.SPLIT = self.OFFP[mid]


def _po_col(qb):
    return (qb // 3) * 512 + (qb % 3) * AUGW


def build_nc(fp=80, reps=1):
    from concourse.bacc import Bacc

    L = Layout(fp)
    FP, NPK, TOTP = L.FP, L.NPK, L.TOTP

    nc = Bacc()
    qt_d = nc.declare_dram_parameter("qT", [D, S], BF16, isOutput=False)
    kt_d = nc.declare_dram_parameter("kT", [D, S], BF16, isOutput=False)
    vp1_d = nc.declare_dram_parameter("vp1", [NPK, D], BF16, isOutput=False)
    vp2_d = nc.declare_dram_parameter("vp2", [NPK, D], BF16, isOutput=False)
    cmp_d = nc.declare_dram_parameter("cmp", [1, NPK], F32, isOutput=False)
    tric_d = nc.declare_dram_parameter("tricp", [P, S], BF16, isOutput=False)
    addr_d = nc.declare_dram_parameter("addr", [1, H * 9 * AUGW], BF16,
                                       isOutput=False)
    o1_d = nc.declare_dram_parameter("out1", [S, D], BF16, isOutput=True)
    o2_d = nc.declare_dram_parameter("out2", [S, D], BF16, isOutput=True)

    from contextlib import ExitStack

    with TileContext(nc) as tc, ExitStack() as ctx:
        const = ctx.enter_context(tc.tile_pool(name="const", bufs=1))
        qkpool = ctx.enter_context(tc.tile_pool(name="qk", bufs=3))
        epool = ctx.enter_context(tc.tile_pool(name="ep", bufs=16))
        ppep = ctx.enter_context(tc.tile_pool(name="pe2", bufs=2))
        etp = ctx.enter_context(tc.tile_pool(name="et", bufs=2))
        smol = ctx.enter_context(tc.tile_pool(name="sm", bufs=6))
        outp = ctx.enter_context(tc.tile_pool(name="op", bufs=2))
        bigp = ctx.enter_context(tc.tile_pool(name="big", bufs=1))
        # PSUM budget (8 banks): ps 2x2 + po 1x3 = 7
        ps_pool = ctx.enter_context(tc.tile_pool(name="ps", bufs=2,
                                                 space="PSUM"))
        po_pool = ctx.enter_context(tc.tile_pool(name="po", bufs=1,
                                                 space="PSUM"))

        # ---------------- constants ----------------
        warm = const.tile([1, 1], F32, tag="warm")
        nc.gpsimd.memset(warm[:], 0.0)
        nc.scalar.activation(out=warm[:], in_=warm[:],
                             func=mybir.ActivationFunctionType.Exp)

        ident = const.tile([P, P], BF16, tag="ident")
        nc.gpsimd.memset(ident[:], 0.0)
        nc.gpsimd.affine_select(
            out=ident[:], in_=ident[:], compare_op=AluOpType.not_equal,
            fill=1.0, base=0, pattern=[[-1, P]], channel_multiplier=1)

        ones_row = const.tile([1, P], BF16, tag="onesrow")
        nc.gpsimd.memset(ones_row[:], 1.0)

        tricp = const.tile([P, S], BF16, tag="tricp")
        nc.sync.dma_start(out=tricp[:], in_=tric_d[:])
        addr_sb = const.tile([1, H * 9 * AUGW], BF16, tag="addr")
        nc.sync.dma_start(out=addr_sb[:], in_=addr_d[:])

        # counter-mask broadcast [128, NPK] (bf16)
        cmrow = const.tile([1, NPK], F32, tag="cmrow")
        nc.sync.dma_start(out=cmrow[:], in_=cmp_d[:])
        cmrow16 = const.tile([1, NPK], BF16, tag="cmrow16")
        nc.gpsimd.tensor_copy(cmrow16[:], cmrow[:])
        cmbp = const.tile([P, NPK], BF16, tag="cmbp")
        ps_cm = ps_pool.tile([P, 1024], F32, tag="ps")
        for lo in range(0, NPK, 512):
            hi = min(lo + 512, NPK)
            nc.tensor.matmul(ps_cm[:, lo:hi], ones_row[:], cmrow16[:, lo:hi],
                             start=True, stop=True)
        nc.vector.tensor_copy(cmbp[:], ps_cm[:, 0:NPK])

        # vph slots (manual 3-way rotation): [128, chunks, AUGW]
        NVC = NPK // P  # packed v chunks
        vph_tiles = []
        for i in range(3):
            t = const.tile([P, NVC * AUGW], BF16, tag=f"vph{i}")
            t3 = t.rearrange("p (c w) -> p c w", w=AUGW)
            nc.gpsimd.memset(t3[:, :, 2 * DK:2 * DK + 1], 1.0)
            nc.gpsimd.memset(t3[:, :, 2 * DK + 1:AUGW], 0.0)
            vph_tiles.append(t3)

        # pp buffers with constant PADVAL in pad regions (stt never writes
        # them; exp2 is out-of-place so they survive)
        pp_tiles = []
        for i in range(2):
            t = const.tile([P, TOTP], BF16, tag=f"pp{i}")
            nc.gpsimd.memset(t[:], PADVAL)
            pp_tiles.append(t)

        big1 = bigp.tile([P, NQB * S], BF16, tag="big1")
        big2 = bigp.tile([P, NQB * S], BF16, tag="big2")

        # ------------- main loop: 16 heads, 3-stage software pipeline ------
        state = {}

        def stage_load(hp):
            if hp >= NQB or ("pair", hp) in state:
                return
            dsl = slice(hp * P, (hp + 1) * P)
            qT2 = qkpool.tile([P, S], BF16, tag="qT2")
            kT2 = qkpool.tile([P, S], BF16, tag="kT2")
            # halves so the first scores don't wait on the full row; the
            # first pair in finer pieces so qb0 scores start asap
            cuts = [0, 128, 512, S] if hp == 0 else [0, 512, S]
            for lo, hi in zip(cuts, cuts[1:]):
                nc.sync.dma_start(out=kT2[:, lo:hi], in_=kt_d[dsl, lo:hi])
                nc.sync.dma_start(out=qT2[:, lo:hi], in_=qt_d[dsl, lo:hi])
            state[("pair", hp)] = (qT2, kT2)

        def stage_loadv(h):
            if h >= H or ("vph", h) in state:
                return
            hsl = slice(h * DK, (h + 1) * DK)
            vph = vph_tiles[h % 3]
            nc.sync.dma_start(
                out=vph[:, :, 0:DK],
                in_=vp1_d[:, hsl].rearrange("(c p) d -> p c d", p=P))
            nc.sync.dma_start(
                out=vph[:, :, DK:2 * DK],
                in_=vp2_d[:, hsl].rearrange("(c p) d -> p c d", p=P))
            state[("vph", h)] = vph

        def stage_a(h):
            hp, hl = divmod(h, 2)
            stage_load(hp)
            stage_loadv(h)
            pb = hl * DK
            z1 = smol.tile([P, NQB], F32, tag="z1")
            state[h] = dict(pb=pb, z1=z1, etiles=[])
            _score_exp1(h, range(0, 6))

        def _score_exp1(h, qbs):
            st = state[h]
            hp, hl = divmod(h, 2)
            qT2, kT2 = state[("pair", hp)]
            pb, z1, etiles = st["pb"], st["z1"], st["etiles"]
            for qb in qbs:
                W = (qb + 1) * P
                ps = ps_pool.tile([P, S], F32, tag="ps")
                for lo in range(0, W, 512):
                    hi = min(lo + 512, W)
                    nc.tensor.matmul(
                        ps[:, lo:hi],
                        qT2[pb:pb + DK, qb * P:(qb + 1) * P],
                        kT2[pb:pb + DK, lo:hi],
                        start=True, stop=(hi < W))
                nc.tensor.matmul(
                    ps[:, W - P:W], ident[:],
                    tricp[:, qb * P:(qb + 1) * P],
                    start=False, stop=True)
                e_t = epool.tile([P, S], BF16, tag="E")
                nc.scalar.activation(
                    out=e_t[:, 0:W], in_=ps[:, 0:W],
                    func=mybir.ActivationFunctionType.Exp,
                    scale=0.125, accum_out=z1[:, qb:qb + 1])
                etiles.append(e_t)

        def stage_a2(h):
            _score_exp1(h, range(6, NQB))

        def stage_b1(h):
            st = state[h]
            r1 = smol.tile([P, NQB], F32, tag="r1")
            nc.vector.reciprocal(r1[:], st["z1"][:])
            nc.vector.memset(r1[0:1, 0:1], 0.0)  # query row 0: zero out

            pp = pp_tiles[h % 2]
            for qb in range(NQB):
                lr = L.LREAL[qb]
                ev = st["etiles"][qb].rearrange(
                    "p (c x) -> p c x", x=P)[:, 0:qb + 1, 0:FP]
                cv = cmbp[:, 0:lr].rearrange("p (c x) -> p c x", x=FP)
                ov = pp[:, L.OFFP[qb]:L.OFFP[qb] + lr].rearrange(
                    "p (c x) -> p c x", x=FP)
                nc.vector.scalar_tensor_tensor(
                    out=ov, in0=ev, scalar=r1[:, qb:qb + 1], in1=cv,
                    op0=AluOpType.mult, op1=AluOpType.mult)
            st["pp"] = pp

        def stage_b2a(h):
            st = state[h]
            pp = st["pp"]
            ppe = ppep.tile([P, TOTP], BF16, tag="ppe")
            nc.scalar.activation(out=ppe[:, 0:L.SPLIT], in_=pp[:, 0:L.SPLIT],
                                 func=mybir.ActivationFunctionType.Exp)
            nc.scalar.activation(out=ppe[:, L.SPLIT:], in_=pp[:, L.SPLIT:],
                                 func=mybir.ActivationFunctionType.Exp)
            st["ppe"] = ppe

        def stage_b2b(h):
            st = state[h]
            ppe = st["ppe"]
            vph = state.pop(("vph", h))

            e2t = etp.tile([P, TOTP], BF16, tag="e2t")
            NB = L.SPLIT // P
            nc.sync.dma_start(
                out=e2t[:, 0:L.SPLIT].rearrange("p (n s) -> p n s", n=NB),
                in_=ppe[:, 0:L.SPLIT].rearrange("p (n s) -> p n s", n=NB),
                transpose=True)
            nc.sync.dma_start(
                out=e2t[:, L.SPLIT:].rearrange("p (n s) -> p n s",
                                               n=TOTP // P - NB),
                in_=ppe[:, L.SPLIT:].rearrange("p (n s) -> p n s",
                                               n=TOTP // P - NB),
                transpose=True)

            # each 512-f32 po bank: one start (first write), one stop (last)
            po = po_pool.tile([P, POW], F32, tag="po")
            for qb in range(NQB):
                col = _po_col(qb)
                aslot = (h * 9 + qb) * AUGW
                nc.tensor.matmul(
                    po[:, col:col + AUGW], ones_row[:],
                    addr_sb[0:1, aslot:aslot + AUGW],
                    start=(qb % 3 == 0), stop=False)
                nchunks = L.LPAD[qb] // P
                for n in range(nchunks):
                    g = L.OFFP[qb] // P + n
                    nc.tensor.matmul(
                        po[:, col:col + AUGW],
                        e2t[:, g * P:(g + 1) * P],
                        vph[:, n, :],
                        start=False,
                        stop=(qb % 3 == 2 and n == nchunks - 1))
            # dummy 9th block fills the pad slot of bank 2 (finite for recip);
            # last write into bank 2 -> carries its stop
            aslot = (h * 9 + 8) * AUGW
            nc.tensor.matmul(
                po[:, 2 * 512 + 2 * AUGW:2 * 512 + 3 * AUGW], ones_row[:],
                addr_sb[0:1, aslot:aslot + AUGW],
                start=False, stop=True)
            st["po"] = po

        def stage_c(h):
            st = state.pop(h)
            po = st["po"]
            r2 = smol.tile([P, 9], F32, tag="r2")
            for bk in range(3):
                sub = po[:, bk * 512:bk * 512 + 3 * AUGW].rearrange(
                    "p (z y) -> p z y", z=3)
                nc.vector.reciprocal(
                    r2[:, 3 * bk:3 * bk + 3],
                    sub[:, :, 2 * DK:2 * DK + 1].rearrange(
                        "p a b -> p (a b)"))
            b13 = big1.rearrange("p (c d) -> p c d", c=NQB)
            b23 = big2.rearrange("p (c d) -> p c d", c=NQB)
            hc = slice(h * DK, (h + 1) * DK)
            for qb in range(NQB):
                col = _po_col(qb)
                nc.vector.tensor_scalar_mul(
                    b13[:, qb, hc], po[:, col:col + DK], r2[:, qb:qb + 1])
                nc.vector.tensor_scalar_mul(
                    b23[:, qb, hc], po[:, col + DK:col + 2 * DK],
                    r2[:, qb:qb + 1])
            nc.gpsimd.memset(big1[0:1, h * DK:(h + 1) * DK], 0.0)
            nc.gpsimd.memset(big2[0:1, h * DK:(h + 1) * DK], 0.0)
            if h % 4 == 3:
                g = slice((h - 3) * DK, (h + 1) * DK)
                nc.sync.dma_start(
                    out=o1_d[:, g].rearrange("(c s) d -> s c d", c=NQB),
                    in_=b13[:, :, g])
                nc.sync.dma_start(
                    out=o2_d[:, g].rearrange("(c s) d -> s c d", c=NQB),
                    in_=b23[:, :, g])

        for _rep in range(reps):
            state.clear()
            for it in range(H + 2):
                if it < H:
                    stage_a(it)
                if 1 <= it <= H:
                    stage_b1(it - 1)
                    stage_b2a(it - 1)
                if it < H:
                    stage_a2(it)
                if 1 <= it <= H:
                    stage_b2b(it - 1)
                if it >= 2:
                    stage_c(it - 2)
                if it < H:
                    stage_loadv(it + 1)
                    stage_load(it // 2 + 1)
                    stage_load(it // 2 + 2)
    nc.compile()
    return nc


_NC_CACHE = {}


def _get_nc(fp):
    if fp not in _NC_CACHE:
        _NC_CACHE[fp] = build_nc(fp=fp)
    return _NC_CACHE[fp]


def pick_fp(cm):
    """Smallest multiple of 16 (>=64) covering every chunk's unmasked count."""
    u = (np.asarray(cm).reshape(NCORES, NQB, P) == 0).sum(-1).max()
    return max(64, int(-(-int(u + 1) // 16) * 16))


def prep_inputs(q, k, v1, v2, counter_attention_mask, fp=None):
    """Host-side shard prep: per-chunk permutation (unmasked keys first),
    packed-FP v/cm, per-batch causal addend, correction rows, bf16 casts."""
    import ml_dtypes

    bf = ml_dtypes.bfloat16
    q = np.asarray(q, dtype=np.float32)
    k = np.asarray(k, dtype=np.float32)
    v1 = np.asarray(v1, dtype=np.float32)
    v2 = np.asarray(v2, dtype=np.float32)
    cm = np.asarray(counter_attention_mask)
    if fp is None:
        fp = pick_fp(cm)
    L = Layout(fp)

    pref = np.concatenate([np.arange(kc * P, kc * P + fp)
                           for kc in range(NQB)])
    maps = []
    for b in range(NCORES):
        perm = np.concatenate([
            kc * P + np.argsort(cm[b, kc * P:(kc + 1) * P], kind="stable")
            for kc in range(NQB)
        ])
        kp = k[b][perm]
        v1p, v2p = v1[b][perm], v2[b][perm]
        vp1, vp2 = v1p[pref], v2p[pref]                  # [NPK, D]
        cmpk = (cm[b][perm][pref] == 0).astype(np.float32)[None]  # [1,NPK]

        tricp = np.zeros((P, S), np.float32)
        for qb in range(NQB):
            old = perm[qb * P:(qb + 1) * P] - qb * P
            tricp[:, qb * P:(qb + 1) * P] = np.where(
                old[None, :] < np.arange(P)[:, None], 0.0, MASKADD)

        addr = np.zeros((H, 9, AUGW), np.float32)
        t1, t2 = v1[b].sum(0), v2[b].sum(0)              # [D]
        c1 = np.cumsum(vp1.reshape(NQB, fp, D).sum(1), 0)  # prefix cs [8, D]
        c2 = np.cumsum(vp2.reshape(NQB, fp, D).sum(1), 0)
        for h in range(H):
            hsl = slice(h * DK, (h + 1) * DK)
            for qb in range(NQB):
                addr[h, qb, 0:DK] = t1[hsl] - c1[qb, hsl]
                addr[h, qb, DK:2 * DK] = t2[hsl] - c2[qb, hsl]
                addr[h, qb, 2 * DK] = float(S - L.LREAL[qb])
            addr[h, 8] = addr[h, 7]
        maps.append({
            "qT": np.ascontiguousarray(q[b].astype(bf).T),
            "kT": np.ascontiguousarray(kp.astype(bf).T),
            "vp1": vp1.astype(bf), "vp2": vp2.astype(bf),
            "cmp": cmpk,
            "tricp": tricp.astype(bf),
            "addr": addr.reshape(1, -1).astype(bf),
        })
    return maps


def kernel(q, k, v1, v2, counter_attention_mask):
    from concourse.bass_utils import run_bass_kernel_spmd

    fp = pick_fp(counter_attention_mask)
    in_maps = prep_inputs(q, k, v1, v2, counter_attention_mask, fp=fp)
    nc = _get_nc(fp)
    res = run_bass_kernel_spmd(nc, in_maps, list(range(NCORES))).results
    out1 = np.stack([res[b]["out1"].astype(np.float32)
                     for b in range(NCORES)])
    out2 = np.stack([res[b]["out2"].astype(np.float32)
                     for b in range(NCORES)])
    return out1, out2


# revision 5
# speedup vs baseline: 1.5840x; 1.1735x over previous
"""DualAttention Trainium2 kernel v2 (8 NeuronCores, data-parallel over batch).

Math per batch/head (dk=64, S=1024), with per-chunk key permutation putting
counter-mask-unmasked keys first (host-side):
  E   = exp(q@kp^T/8) windowed (strict causal via per-batch permuted tric)
  Z1  = rowsum(E) over the full window (ACT accum)
  p1  = (E/Z1)*w on the FP-wide packed prefix of each 128-chunk only
        (FP chosen at runtime >= max unmasked/chunk; w zeroes masked/pad
        slots, pad cols hold -100 so exp2 -> 0)
  E2  = exp(p1) on packed groups (padded to 128-aligned per-qb groups)
  out = (E2 @ [vp1|vp2|1] + ones x addrow_qb) / Z2
        addrow_qb = [colsum(v) - colsum(vp[:L]), S - L]  (host-precomputed;
        folds the E2=1 background for masked/future keys AND Z2's constant)
  Z2  = aug column of the same PSUM accumulation. Row 0 forced to 0.

vs v1: ~-30% exp2/mask/transpose/PV work, ~60 fewer small matmuls per head
(pz/stairs/colsum machinery folded into the augmented matmul + host rows),
bf16 outputs (half the output DMA traffic), DVE queue no longer blocked by
the 1/Z2 reciprocals (B1 emitted before C).
"""

import numpy as np

import concourse.bass as bass
import concourse.mybir as mybir
from concourse.tile import TileContext
from concourse.alu_op_type import AluOpType

F32 = mybir.dt.float32
BF16 = mybir.dt.bfloat16
FP8 = mybir.dt.float8e4

B, S, D = 8, 1024, 1024
H, DK = 16, 64
NCORES = 8
P = 128          # partition block
NQB = S // P     # 8 query blocks
AUGW = 130       # per-qb PV output block: v1(64) | v2(64) | z2(1) | pad(1)
MASKADD = -1e30
PADVAL = -100.0  # exp2(pad) == 0
POW = 1536       # po: 3 qb-blocks of AUGW per 512-f32 PSUM bank


class Layout:
    """Packed-prefix layout for a given prefix width FP (multiple of 16)."""

    def __init__(self, fp):
        self.FP = fp
        self.NPK = NQB * fp
        self.LREAL = [fp * (qb + 1) for qb in range(NQB)]
        self.LPAD = [((l + P - 1) // P) * P for l in self.LREAL]
        self.OFFP = [0]
        for l in self.LPAD:
            self.OFFP.append(self.OFFP[-1] + l)
        self.TOTP = self.OFFP[-1]
        # exp2/transpose split point: group boundary near the middle,
        # 128-chunk aligned
        mid = min(range(1, NQB), key=lambda i: abs(self.OFFP[i] - self.TOTP // 2))
        self.SPLIT = self.OFFP[mid]


def _po_col(qb):
    return (qb // 3) * 512 + (qb % 3) * AUGW


def build_nc(fp=80, reps=1):
    from concourse.bacc import Bacc

    L = Layout(fp)
    FP, NPK, TOTP = L.FP, L.NPK, L.TOTP

    nc = Bacc()
    qt_d = nc.declare_dram_parameter("qT", [D, S], FP8, isOutput=False)
    kt_d = nc.declare_dram_parameter("kT", [D, S], FP8, isOutput=False)
    vp1_d = nc.declare_dram_parameter("vp1", [NPK, D], BF16, isOutput=False)
    vp2_d = nc.declare_dram_parameter("vp2", [NPK, D], BF16, isOutput=False)
    cmp_d = nc.declare_dram_parameter("cmp", [1, NPK], F32, isOutput=False)
    tric_d = nc.declare_dram_parameter("tricp", [P, S], BF16, isOutput=False)
    addr_d = nc.declare_dram_parameter("addr", [1, H * 9 * AUGW], BF16,
                                       isOutput=False)
    o1_d = nc.declare_dram_parameter("out1", [S, D], BF16, isOutput=True)
    o2_d = nc.declare_dram_parameter("out2", [S, D], BF16, isOutput=True)

    from contextlib import ExitStack

    with TileContext(nc) as tc, ExitStack() as ctx:
        const = ctx.enter_context(tc.tile_pool(name="const", bufs=1))
        qkpool = ctx.enter_context(tc.tile_pool(name="qk", bufs=3))
        epool = ctx.enter_context(tc.tile_pool(name="ep", bufs=16))
        ppep = ctx.enter_context(tc.tile_pool(name="pe2", bufs=2))
        etp = ctx.enter_context(tc.tile_pool(name="et", bufs=2))
        smol = ctx.enter_context(tc.tile_pool(name="sm", bufs=6))
        outp = ctx.enter_context(tc.tile_pool(name="op", bufs=2))
        bigp = ctx.enter_context(tc.tile_pool(name="big", bufs=1))
        # PSUM budget (8 banks): ps 2x2 + po 1x3 = 7
        ps_pool = ctx.enter_context(tc.tile_pool(name="ps", bufs=2,
                                                 space="PSUM"))
        po_pool = ctx.enter_context(tc.tile_pool(name="po", bufs=1,
                                                 space="PSUM"))

        # ---------------- constants ----------------
        warm = const.tile([1, 1], F32, tag="warm")
        nc.gpsimd.memset(warm[:], 0.0)
        nc.scalar.activation(out=warm[:], in_=warm[:],
                             func=mybir.ActivationFunctionType.Exp)

        ident = const.tile([P, P], BF16, tag="ident")
        nc.gpsimd.memset(ident[:], 0.0)
        nc.gpsimd.affine_select(
            out=ident[:], in_=ident[:], compare_op=AluOpType.not_equal,
            fill=1.0, base=0, pattern=[[-1, P]], channel_multiplier=1)

        ones_row = const.tile([1, P], BF16, tag="onesrow")
        nc.gpsimd.memset(ones_row[:], 1.0)

        tricp = const.tile([P, S], BF16, tag="tricp")
        nc.sync.dma_start(out=tricp[:], in_=tric_d[:])
        addr_sb = const.tile([1, H * 9 * AUGW], BF16, tag="addr")
        nc.sync.dma_start(out=addr_sb[:], in_=addr_d[:])

        # counter-mask broadcast [128, NPK] (bf16)
        cmrow = const.tile([1, NPK], F32, tag="cmrow")
        nc.sync.dma_start(out=cmrow[:], in_=cmp_d[:])
        cmrow16 = const.tile([1, NPK], BF16, tag="cmrow16")
        nc.gpsimd.tensor_copy(cmrow16[:], cmrow[:])
        cmbp = const.tile([P, NPK], BF16, tag="cmbp")
        ps_cm = ps_pool.tile([P, 1024], F32, tag="ps")
        for lo in range(0, NPK, 512):
            hi = min(lo + 512, NPK)
            nc.tensor.matmul(ps_cm[:, lo:hi], ones_row[:], cmrow16[:, lo:hi],
                             start=True, stop=True)
        nc.vector.tensor_copy(cmbp[:], ps_cm[:, 0:NPK])

        # vph slots (manual 3-way rotation): [128, chunks, AUGW]
        NVC = NPK // P  # packed v chunks
        vph_tiles = []
        for i in range(3):
            t = const.tile([P, NVC * AUGW], BF16, tag=f"vph{i}")
            t3 = t.rearrange("p (c w) -> p c w", w=AUGW)
            nc.gpsimd.memset(t3[:, :, 2 * DK:2 * DK + 1], 1.0)
            nc.gpsimd.memset(t3[:, :, 2 * DK + 1:AUGW], 0.0)
            vph_tiles.append(t3)

        # pp buffers with constant PADVAL in pad regions (stt never writes
        # them; exp2 is out-of-place so they survive)
        pp_tiles = []
        for i in range(2):
            t = const.tile([P, TOTP], BF16, tag=f"pp{i}")
            nc.gpsimd.memset(t[:], PADVAL)
            pp_tiles.append(t)

        big1 = bigp.tile([P, NQB * S], BF16, tag="big1")
        big2 = bigp.tile([P, NQB * S], BF16, tag="big2")

        # ------------- main loop: 16 heads, 3-stage software pipeline ------
        state = {}

        def stage_load(hp):
            if hp >= NQB or ("pair", hp) in state:
                return
            dsl = slice(hp * P, (hp + 1) * P)
            qT2 = qkpool.tile([P, S], FP8, tag="qT2")
            kT2 = qkpool.tile([P, S], FP8, tag="kT2")
            # halves so the first scores don't wait on the full row; the
            # first pair in finer pieces so qb0 scores start asap
            cuts = [0, 128, 512, S] if hp == 0 else [0, 512, S]
            for lo, hi in zip(cuts, cuts[1:]):
                nc.sync.dma_start(out=kT2[:, lo:hi], in_=kt_d[dsl, lo:hi])
                nc.sync.dma_start(out=qT2[:, lo:hi], in_=qt_d[dsl, lo:hi])
            state[("pair", hp)] = (qT2, kT2)

        def stage_loadv(h):
            if h >= H or ("vph", h) in state:
                return
            hsl = slice(h * DK, (h + 1) * DK)
            vph = vph_tiles[h % 3]
            nc.sync.dma_start(
                out=vph[:, :, 0:DK],
                in_=vp1_d[:, hsl].rearrange("(c p) d -> p c d", p=P))
            nc.sync.dma_start(
                out=vph[:, :, DK:2 * DK],
                in_=vp2_d[:, hsl].rearrange("(c p) d -> p c d", p=P))
            state[("vph", h)] = vph

        def stage_a(h):
            hp, hl = divmod(h, 2)
            stage_load(hp)
            stage_loadv(h)
            pb = hl * DK
            z1 = smol.tile([P, NQB], F32, tag="z1")
            state[h] = dict(pb=pb, z1=z1, etiles=[])
            _score_exp1(h, range(0, 6))

        def _score_exp1(h, qbs):
            st = state[h]
            hp, hl = divmod(h, 2)
            qT2, kT2 = state[("pair", hp)]
            pb, z1, etiles = st["pb"], st["z1"], st["etiles"]
            for qb in qbs:
                W = (qb + 1) * P
                ps = ps_pool.tile([P, S], F32, tag="ps")
                for lo in range(0, W, 512):
                    hi = min(lo + 512, W)
                    nc.tensor.matmul(
                        ps[:, lo:hi],
                        qT2[pb:pb + DK, qb * P:(qb + 1) * P],
                        kT2[pb:pb + DK, lo:hi],
                        start=True, stop=(hi < W))
                nc.tensor.matmul(
                    ps[:, W - P:W], ident[:],
                    tricp[:, qb * P:(qb + 1) * P],
                    start=False, stop=True)
                e_t = epool.tile([P, S], BF16, tag="E")
                nc.scalar.activation(
                    out=e_t[:, 0:W], in_=ps[:, 0:W],
                    func=mybir.ActivationFunctionType.Exp,
                    scale=0.125, accum_out=z1[:, qb:qb + 1])
                etiles.append(e_t)

        def stage_a2(h):
            _score_exp1(h, range(6, NQB))

        def stage_b1(h):
            st = state[h]
            r1 = smol.tile([P, NQB], F32, tag="r1")
            nc.vector.reciprocal(r1[:], st["z1"][:])
            nc.vector.memset(r1[0:1, 0:1], 0.0)  # query row 0: zero out

            pp = pp_tiles[h % 2]
            for qb in range(NQB):
                lr = L.LREAL[qb]
                ev = st["etiles"][qb].rearrange(
                    "p (c x) -> p c x", x=P)[:, 0:qb + 1, 0:FP]
                cv = cmbp[:, 0:lr].rearrange("p (c x) -> p c x", x=FP)
                ov = pp[:, L.OFFP[qb]:L.OFFP[qb] + lr].rearrange(
                    "p (c x) -> p c x", x=FP)
                nc.vector.scalar_tensor_tensor(
                    out=ov, in0=ev, scalar=r1[:, qb:qb + 1], in1=cv,
                    op0=AluOpType.mult, op1=AluOpType.mult)
            st["pp"] = pp

        def stage_b2a(h):
            st = state[h]
            pp = st["pp"]
            ppe = ppep.tile([P, TOTP], BF16, tag="ppe")
            nc.scalar.activation(out=ppe[:, 0:L.SPLIT], in_=pp[:, 0:L.SPLIT],
                                 func=mybir.ActivationFunctionType.Exp)
            nc.scalar.activation(out=ppe[:, L.SPLIT:], in_=pp[:, L.SPLIT:],
                                 func=mybir.ActivationFunctionType.Exp)
            st["ppe"] = ppe

        def stage_b2b(h):
            st = state[h]
            ppe = st["ppe"]
            vph = state.pop(("vph", h))

            e2t = etp.tile([P, TOTP], BF16, tag="e2t")
            NB = L.SPLIT // P
            nc.sync.dma_start(
                out=e2t[:, 0:L.SPLIT].rearrange("p (n s) -> p n s", n=NB),
                in_=ppe[:, 0:L.SPLIT].rearrange("p (n s) -> p n s", n=NB),
                transpose=True)
            nc.sync.dma_start(
                out=e2t[:, L.SPLIT:].rearrange("p (n s) -> p n s",
                                               n=TOTP // P - NB),
                in_=ppe[:, L.SPLIT:].rearrange("p (n s) -> p n s",
                                               n=TOTP // P - NB),
                transpose=True)

            # each 512-f32 po bank: one start (first write), one stop (last)
            po = po_pool.tile([P, POW], F32, tag="po")
            for qb in range(NQB):
                col = _po_col(qb)
                aslot = (h * 9 + qb) * AUGW
                nc.tensor.matmul(
                    po[:, col:col + AUGW], ones_row[:],
                    addr_sb[0:1, aslot:aslot + AUGW],
                    start=(qb % 3 == 0), stop=False)
                nchunks = L.LPAD[qb] // P
                for n in range(nchunks):
                    g = L.OFFP[qb] // P + n
                    nc.tensor.matmul(
                        po[:, col:col + AUGW],
                        e2t[:, g * P:(g + 1) * P],
                        vph[:, n, :],
                        start=False,
                        stop=(qb % 3 == 2 and n == nchunks - 1))
            # dummy 9th block fills the pad slot of bank 2 (finite for recip);
            # last write into bank 2 -> carries its stop
            aslot = (h * 9 + 8) * AUGW
            nc.tensor.matmul(
                po[:, 2 * 512 + 2 * AUGW:2 * 512 + 3 * AUGW], ones_row[:],
                addr_sb[0:1, aslot:aslot + AUGW],
                start=False, stop=True)
            st["po"] = po

        def stage_c(h):
            st = state.pop(h)
            po = st["po"]
            r2 = smol.tile([P, 9], F32, tag="r2")
            for bk in range(3):
                sub = po[:, bk * 512:bk * 512 + 3 * AUGW].rearrange(
                    "p (z y) -> p z y", z=3)
                nc.vector.reciprocal(
                    r2[:, 3 * bk:3 * bk + 3],
                    sub[:, :, 2 * DK:2 * DK + 1].rearrange(
                        "p a b -> p (a b)"))
            b13 = big1.rearrange("p (c d) -> p c d", c=NQB)
            b23 = big2.rearrange("p (c d) -> p c d", c=NQB)
            hc = slice(h * DK, (h + 1) * DK)
            for qb in range(NQB):
                col = _po_col(qb)
                nc.vector.tensor_scalar_mul(
                    b13[:, qb, hc], po[:, col:col + DK], r2[:, qb:qb + 1])
                nc.vector.tensor_scalar_mul(
                    b23[:, qb, hc], po[:, col + DK:col + 2 * DK],
                    r2[:, qb:qb + 1])
            nc.gpsimd.memset(big1[0:1, h * DK:(h + 1) * DK], 0.0)
            nc.gpsimd.memset(big2[0:1, h * DK:(h + 1) * DK], 0.0)
            if h % 4 == 3:
                g = slice((h - 3) * DK, (h + 1) * DK)
                nc.sync.dma_start(
                    out=o1_d[:, g].rearrange("(c s) d -> s c d", c=NQB),
                    in_=b13[:, :, g])
                nc.sync.dma_start(
                    out=o2_d[:, g].rearrange("(c s) d -> s c d", c=NQB),
                    in_=b23[:, :, g])

        for _rep in range(reps):
            state.clear()
            for it in range(H + 2):
                if it < H:
                    stage_a(it)
                if 1 <= it <= H:
                    stage_b1(it - 1)
                    stage_b2a(it - 1)
                if it < H:
                    stage_a2(it)
                if 1 <= it <= H:
                    stage_b2b(it - 1)
                if it >= 2:
                    stage_c(it - 2)
                if it < H:
                    stage_loadv(it + 1)
                    stage_load(it // 2 + 1)
                    stage_load(it // 2 + 2)
    nc.compile()
    return nc


_NC_CACHE = {}


def _get_nc(fp):
    if fp not in _NC_CACHE:
        _NC_CACHE[fp] = build_nc(fp=fp)
    return _NC_CACHE[fp]


def pick_fp(cm):
    """Smallest multiple of 16 (>=64) covering every chunk's unmasked count."""
    u = (np.asarray(cm).reshape(NCORES, NQB, P) == 0).sum(-1).max()
    return max(64, int(-(-int(u + 1) // 16) * 16))


def prep_inputs(q, k, v1, v2, counter_attention_mask, fp=None):
    """Host-side shard prep: per-chunk permutation (unmasked keys first),
    packed-FP v/cm, per-batch causal addend, correction rows, bf16 casts."""
    import ml_dtypes

    bf = ml_dtypes.bfloat16
    f8 = ml_dtypes.float8_e4m3
    q = np.asarray(q, dtype=np.float32)
    k = np.asarray(k, dtype=np.float32)
    v1 = np.asarray(v1, dtype=np.float32)
    v2 = np.asarray(v2, dtype=np.float32)
    cm = np.asarray(counter_attention_mask)
    if fp is None:
        fp = pick_fp(cm)
    L = Layout(fp)

    pref = np.concatenate([np.arange(kc * P, kc * P + fp)
                           for kc in range(NQB)])
    maps = []
    for b in range(NCORES):
        perm = np.concatenate([
            kc * P + np.argsort(cm[b, kc * P:(kc + 1) * P], kind="stable")
            for kc in range(NQB)
        ])
        kp = k[b][perm]
        v1p, v2p = v1[b][perm], v2[b][perm]
        vp1, vp2 = v1p[pref], v2p[pref]                  # [NPK, D]
        cmpk = (cm[b][perm][pref] == 0).astype(np.float32)[None]  # [1,NPK]

        tricp = np.zeros((P, S), np.float32)
        for qb in range(NQB):
            old = perm[qb * P:(qb + 1) * P] - qb * P
            tricp[:, qb * P:(qb + 1) * P] = np.where(
                old[None, :] < np.arange(P)[:, None], 0.0, MASKADD)

        addr = np.zeros((H, 9, AUGW), np.float32)
        t1, t2 = v1[b].sum(0), v2[b].sum(0)              # [D]
        c1 = np.cumsum(vp1.reshape(NQB, fp, D).sum(1), 0)  # prefix cs [8, D]
        c2 = np.cumsum(vp2.reshape(NQB, fp, D).sum(1), 0)
        for h in range(H):
            hsl = slice(h * DK, (h + 1) * DK)
            for qb in range(NQB):
                addr[h, qb, 0:DK] = t1[hsl] - c1[qb, hsl]
                addr[h, qb, DK:2 * DK] = t2[hsl] - c2[qb, hsl]
                addr[h, qb, 2 * DK] = float(S - L.LREAL[qb])
            addr[h, 8] = addr[h, 7]
        maps.append({
            "qT": np.ascontiguousarray(q[b].astype(f8).T),
            "kT": np.ascontiguousarray(kp.astype(f8).T),
            "vp1": vp1.astype(bf), "vp2": vp2.astype(bf),
            "cmp": cmpk,
            "tricp": tricp.astype(bf),
            "addr": addr.reshape(1, -1).astype(bf),
        })
    return maps


def kernel(q, k, v1, v2, counter_attention_mask):
    from concourse.bass_utils import run_bass_kernel_spmd

    fp = pick_fp(counter_attention_mask)
    in_maps = prep_inputs(q, k, v1, v2, counter_attention_mask, fp=fp)
    nc = _get_nc(fp)
    res = run_bass_kernel_spmd(nc, in_maps, list(range(NCORES))).results
    out1 = np.stack([res[b]["out1"].astype(np.float32)
                     for b in range(NCORES)])
    out2 = np.stack([res[b]["out2"].astype(np.float32)
                     for b in range(NCORES)])
    return out1, out2
